# revision 6
# baseline (speedup 1.0000x reference)
"""Coherence-enhancing diffusion layer on 8 TRN2 NeuronCores.

Data-parallel: one 256x256 image per core, 100 diffusion iterations fully
on-chip.  All-fp32 (the 100-iteration nonlinear diffusion chaotically
amplifies any lower-precision rounding; fp32r matmuls measure ~1.5e-4/pass
which amplifies to ~6e-2 final error -- over the gate).

Structure per iteration (u stored in "R" layout [128, 512]:
tile[p, 256*b + c] = u[128*b + p, c]):
  - Sobel column convs ([-1,0,1] / [1,2,1]) as DVE/Pool free-axis shifts.
  - Sobel row convs as banded 256x256 matmuls on PE -> gx^T, gy^T ("C").
  - Structure tensor products; rho never materializes (blur linearity:
    blur2(blur1(q) - blur1(c*u0)) = blur(q) - c*rho).
  - 3 separable 9-tap Gaussian blurs as banded matmul pass pairs; the
    B (=2*S12) and D (=S11-S22) pass-2 outputs are interleaved per
    column-half into shared PSUM banks [2B | D] so the tail squares both
    with ONE 512-wide ACT op; 4*B^2+D^2 = t^2 follows as one add.
  - Pointwise tail per column-half: a1/a2 written into adjacent halves of
    one [128,512] tile so square/exp/mul run as single 512-wide ops;
    sqrt via exp(0.5*ln(x)) keeps ACT on one LUT set (a real Sqrt forces
    a ~2.7us table swap twice/iter).
  - Engine balance: ACT ~6.8us, DVE ~8.4us, Pool ~7us, PE ~7.5us per
    iteration; scheduling overlap across the two column-half chains and
    across iterations (u updates gate per column-half).
  - delta transposed back to R via 4 PE transpose matmuls; the u update
    reads transposed delta straight from PSUM (also keeps PE warm through
    the pointwise tail).
"""
import numpy as np

N = 256
P = 128
NB = 2
ITERS = 100
RHO_INT = 4
KSIZE = 9
N_CORES = 8

_LUMA = (0.299, 0.587, 0.114)


# ---------------------------------------------------------------- host math
def _gauss1d_f32():
    half = (KSIZE - 1) / 2.0
    xs = np.linspace(-half, half, KSIZE).astype(np.float32)
    t = xs / np.float32(RHO_INT)
    pdf = np.exp(np.float32(-0.5) * t * t).astype(np.float32)
    return (pdf / pdf.sum()).astype(np.float32)


def _band_matrix(taps, pad):
    r = len(taps) // 2
    M = np.zeros((N, N), np.float64)
    for i in range(N):
        for d in range(-r, r + 1):
            j = i + d
            w = taps[d + r]
            if pad == "zero":
                if 0 <= j < N:
                    M[i, j] += w
            else:  # reflect (jnp.pad mode='reflect')
                if j < 0:
                    j = -j
                elif j >= N:
                    j = 2 * (N - 1) - j
                M[i, j] += w
    return M.astype(np.float32)


def _wt_tile(W):
    """Pack W^T into a [128, 512] block-major tile: out[p, 256k+n] = W[n, 128k+p]."""
    out = np.empty((P, NB * N), np.float32)
    for k in range(NB):
        out[:, N * k:N * (k + 1)] = W.T[P * k:P * (k + 1), :]
    return np.ascontiguousarray(out)


def _nz_ranges(W):
    """Per contraction block k: [lo, hi) range of output rows r' with any
    nonzero weight W[r', r], r in block k."""
    rngs = []
    for k in range(NB):
        nz = np.nonzero(np.any(W[:, P * k:P * (k + 1)] != 0, axis=1))[0]
        rngs.append((int(nz.min()), int(nz.max()) + 1))
    return rngs


def _pack_img(img):
    """[256,256] -> [128,512] R-layout."""
    return np.ascontiguousarray(
        img.reshape(NB, P, N).transpose(1, 0, 2).reshape(P, NB * N))


def _unpack_img(tile_arr):
    return tile_arr.reshape(P, NB, N).transpose(1, 0, 2).reshape(N, N)


_G = _band_matrix(_gauss1d_f32().astype(np.float64), "reflect")
_T = _band_matrix([1.0, 2.0, 1.0], "zero")
_D = _band_matrix([-1.0, 0.0, 1.0], "zero")


# ---------------------------------------------------------------- bass build
def _build(dt, k, iters, loop_m=1):
    import concourse.bass as bass  # noqa: F401
    import concourse.tile as tile
    from concourse import bacc, mybir

    F32 = mybir.dt.float32
    AF = mybir.ActivationFunctionType
    OP = mybir.AluOpType

    k2 = np.float32(k) * np.float32(k)
    exp_scale = float(-0.25 / k2)
    upd_scale = float(np.float32(0.5) * np.float32(dt))

    nc = bacc.Bacc("TRN2", target_bir_lowering=False, debug=False)

    # Pin all activations to the one LUT set that holds ln+exp+square+copy
    # ("natural_log_exp_and_others") so the act-table pass hoists a single
    # table load to kernel entry instead of ping-ponging sets (~2.7us each).
    import concourse.bacc as bacc_mod
    _orig_tables = bacc_mod.get_activation_tables

    def _pinned_tables(arch):
        tabs = _orig_tables(arch)
        return {
            name: (funcs if "natural_log_exp" in name else set())
            for name, funcs in tabs.items()
        }

    bacc_mod.get_activation_tables = _pinned_tables

    u0_d = nc.dram_tensor("u0", [P, NB * N], F32, kind="ExternalInput").ap()
    u0c2_d = nc.dram_tensor("u0c2", [P, NB * N], F32, kind="ExternalInput").ap()
    wtt_d = nc.dram_tensor("wtt", [P, NB * N], F32, kind="ExternalInput").ap()
    wtd_d = nc.dram_tensor("wtd", [P, NB * N], F32, kind="ExternalInput").ap()
    wtg_d = nc.dram_tensor("wtg", [P, NB * N], F32, kind="ExternalInput").ap()
    id_d = nc.dram_tensor("ident", [P, P], F32, kind="ExternalInput").ap()
    out_d = nc.dram_tensor("uout", [P, NB * N], F32, kind="ExternalOutput").ap()

    rng_t = _nz_ranges(_T)   # == ranges for D (same band)
    rng_g = _nz_ranges(_G)

    with tile.TileContext(nc) as tc:
        with (
            tc.tile_pool(name="consts", bufs=1) as consts,
            tc.tile_pool(name="upool", bufs=(6 if loop_m > 1 else 3)) as upool,
            tc.tile_pool(name="work", bufs=3) as work,
            tc.tile_pool(name="ps", bufs=8, space="PSUM") as psp,
        ):
            wtt = consts.tile([P, NB * N], F32)
            wtd = consts.tile([P, NB * N], F32)
            wtg = consts.tile([P, NB * N], F32)
            u0c2 = consts.tile([P, NB * N], F32)
            ident = consts.tile([P, P], F32)
            nc.sync.dma_start(wtt[:], wtt_d)
            nc.sync.dma_start(wtd[:], wtd_d)
            nc.sync.dma_start(wtg[:], wtg_d)
            nc.sync.dma_start(u0c2[:], u0c2_d)
            nc.sync.dma_start(ident[:], id_d)

            u_t = upool.tile([P, NB * N], F32, tag="u")
            nc.sync.dma_start(u_t[:], u0_d)
            m1u0d = consts.tile([P, NB * N], F32)

            def emit_band(x_tile, w_tile, rngs, out256, j):
                """(W @ X)^T section j -> out256 ([128,256] PSUM window):
                one 2-matmul accumulation group."""
                (lo0, hi0), (lo1, hi1) = rngs
                lhs0 = x_tile[:, N * 0 + P * j:N * 0 + P * j + P]
                lhs1 = x_tile[:, N * 1 + P * j:N * 1 + P * j + P]
                nc.tensor.matmul(out256[:, lo0:hi0], lhs0,
                                 w_tile[:, 0 + lo0:0 + hi0],
                                 start=True, stop=False)
                nc.tensor.matmul(out256[:, lo1:N], lhs1,
                                 w_tile[:, N + lo1:N + N],
                                 start=False, stop=True)

            def conv_pass(x_tile, w_tile, rngs, psum_tile):
                """psum = (W @ X)^T, both sections; one group per section."""
                for j in range(NB):
                    emit_band(x_tile, w_tile, rngs,
                              psum_tile[:, N * j:N * j + N], j)

            def seg(ap, a, b):
                """[128,512] tile -> AP [128, 2, b-a]: free cols a..b of both
                sections."""
                return ap.rearrange("p (s c) -> p s c", s=NB)[:, :, a:b]

            def halves(ap):
                return [seg(ap, P * h, P * h + P) for h in range(NB)]

            # one-time: m1u0d = blur1(2*u0c); rho subtractions ride on the
            # pass-1 PSUM->SBUF moves via blur linearity.
            m1u_ps = psp.tile([P, NB * N], F32, tag="ps")
            conv_pass(u0c2[:], wtg[:], rng_g, m1u_ps)
            nc.scalar.copy(m1u0d[:], m1u_ps[:])

            # ---- diffusion iterations ----
            import contextlib
            outer = tc.For_i(0, loop_m, 1) if loop_m > 1 else contextlib.nullcontext()
            with outer:
                for it in range(iters):
                    u = u_t[:]
                    # Sobel column convs (free-axis shifts, zero-pad):
                    # ud[c] = u[c+1] - u[c-1];  ut[c] = u[c-1] + 2u[c] + u[c+1]
                    ud = work.tile([P, NB * N], F32)
                    s1 = work.tile([P, NB * N], F32)
                    ut = work.tile([P, NB * N], F32)
                    for sx in range(NB):
                        il, ih = (1, P - 1) if sx == 0 else (P + 1, N - 1)
                        sl, sh = (0, P - 1) if sx == 0 else (P, N - 1)
                        nc.vector.tensor_add(seg(s1[:], sl, sh),
                                             seg(u, sl, sh),
                                             seg(u, sl + 1, sh + 1))
                        nc.gpsimd.tensor_sub(seg(ud[:], il, ih),
                                             seg(u, il + 1, ih + 1),
                                             seg(u, il - 1, ih - 1))
                        nc.vector.tensor_add(seg(ut[:], il, ih),
                                             seg(s1[:], il - 1, ih - 1),
                                             seg(s1[:], il, ih))
                        if sx == 0:
                            nc.gpsimd.tensor_copy(seg(ud[:], 0, 1), seg(u, 1, 2))
                            nc.vector.tensor_add(seg(ut[:], 0, 1), seg(s1[:], 0, 1),
                                                 seg(u, 0, 1))
                        else:
                            nc.gpsimd.tensor_scalar_mul(seg(ud[:], N - 1, N),
                                                        seg(u, N - 2, N - 1), -1.0)
                            nc.vector.tensor_add(seg(ut[:], N - 1, N),
                                                 seg(s1[:], N - 2, N - 1),
                                                 seg(u, N - 1, N))
                    # seam: s1[127]; ud[127..128]; ut[127..128]
                    nc.vector.tensor_add(seg(s1[:], P - 1, P), seg(u, P - 1, P),
                                         seg(u, P, P + 1))
                    nc.gpsimd.tensor_sub(seg(ud[:], P - 1, P + 1),
                                         seg(u, P, P + 2), seg(u, P - 2, P))
                    nc.vector.tensor_add(seg(ut[:], P - 1, P + 1),
                                         seg(s1[:], P - 2, P),
                                         seg(s1[:], P - 1, P + 1))

                    # Sobel row convs on PE -> gx^T, gy^T (C layout).
                    # PSUM dependencies are bank-granular, so each section
                    # gets its own bank: section-0 consumers start without
                    # waiting for section-1 matmuls.  gy first: q12 (the
                    # lead blur's input) waits on the gy_c copy.
                    gy_b = [psp.tile([P, N], F32, tag="ps") for _ in range(NB)]
                    gx_b = [psp.tile([P, N], F32, tag="ps") for _ in range(NB)]
                    for j in range(NB):
                        emit_band(ut[:], wtd[:], rng_t, gy_b[j][:], j)
                        emit_band(ud[:], wtt[:], rng_t, gx_b[j][:], j)

                    # structure tensor entries (C layout), per section
                    gy_c = work.tile([P, NB * N], F32)
                    q12 = work.tile([P, NB * N], F32)
                    q11 = work.tile([P, NB * N], F32)
                    q22 = work.tile([P, NB * N], F32)
                    mq = work.tile([P, NB * N], F32)
                    pq = work.tile([P, NB * N], F32)
                    for sx in range(NB):
                        ss = lambda t_: t_[:, N * sx:N * sx + N]
                        nc.vector.tensor_copy(ss(gy_c), ss(gy_ps))
                        nc.scalar.activation(ss(q11), ss(gx_ps), AF.Square)
                        nc.vector.tensor_mul(ss(q12), ss(gx_ps), ss(gy_c))
                        nc.scalar.activation(ss(q22), ss(gy_c), AF.Square)
                        nc.gpsimd.tensor_sub(ss(mq), ss(q11), ss(q22))
                        nc.gpsimd.tensor_add(ss(pq), ss(q11), ss(q22))

                    # blur pass 1 (B quantity first so its chain leads)
                    m1c_ps = psp.tile([P, NB * N], F32, tag="ps")
                    conv_pass(q12[:], wtg[:], rng_g, m1c_ps)
                    m1b_ps = psp.tile([P, NB * N], F32, tag="ps")
                    conv_pass(mq[:], wtg[:], rng_g, m1b_ps)
                    m1a_ps = psp.tile([P, NB * N], F32, tag="ps")
                    conv_pass(pq[:], wtg[:], rng_g, m1a_ps)
                    # full-width PSUM->SBUF moves with the rho folds:
                    # B path doubled (2*blur1(q12) - blur1(2u0c)) so the tail
                    # squares [2B | D] with a single op per section.
                    m1c_c = work.tile([P, NB * N], F32)
                    m1b_c = work.tile([P, NB * N], F32)
                    m1a_c = work.tile([P, NB * N], F32)
                    nc.vector.scalar_tensor_tensor(
                        m1c_c[:], m1c_ps[:], 2.0, m1u0d[:],
                        op0=OP.mult, op1=OP.subtract)
                    nc.scalar.copy(m1b_c[:], m1b_ps[:])
                    nc.vector.tensor_sub(m1a_c[:], m1a_ps[:], m1u0d[:])

                    # blur pass 2: section-major into shared [2B | D] banks
                    # (bd0 first so the section-0 tail chain starts earliest),
                    # then S into its own bank.
                    bd0_ps = psp.tile([P, NB * N], F32, tag="ps")
                    bd1_ps = psp.tile([P, NB * N], F32, tag="ps")
                    s_ps = psp.tile([P, NB * N], F32, tag="ps")
                    bd = (bd0_ps, bd1_ps)
                    for j in range(NB):
                        emit_band(m1c_c[:], wtg[:], rng_g, bd[j][:, 0:N], j)
                        emit_band(m1b_c[:], wtg[:], rng_g, bd[j][:, N:2 * N], j)
                    for j in range(NB):
                        emit_band(m1a_c[:], wtg[:], rng_g,
                                  s_ps[:, N * j:N * j + N], j)

                    # pointwise tail per column-half sx:
                    #   t^2 = (2B)^2 + D^2; t = exp(.5 ln t^2);
                    #   a12 = [S+t | S-t] (= 2*lam); c12 = exp(-a12^2/(4k^2));
                    #   delta = sum of c12*a12 halves; u += dt/2 * delta^T
                    dr_ps = psp.tile([P, NB * N], F32, tag="ps")
                    u_next = upool.tile([P, NB * N], F32, tag="u")
                    for sx in range(NB):
                        t2p = work.tile([P, NB * N], F32, tag=f"t2p{sx}")
                        t2 = work.tile([P, N], F32, tag=f"t2{sx}")
                        lnt = work.tile([P, N], F32, tag=f"lnt{sx}")
                        tm = work.tile([P, N], F32, tag=f"tm{sx}")
                        a12 = work.tile([P, NB * N], F32, tag=f"a12{sx}")
                        sq12 = work.tile([P, NB * N], F32, tag=f"sq12{sx}")
                        c12 = work.tile([P, NB * N], F32, tag=f"c12{sx}")
                        f12 = work.tile([P, NB * N], F32, tag=f"f12{sx}")
                        delta = work.tile([P, N], F32, tag=f"delta{sx}")

                        sps_s = s_ps[:, N * sx:N * sx + N]
                        nc.scalar.activation(t2p[:], bd[sx][:], AF.Square)
                        nc.gpsimd.tensor_add(t2[:], t2p[:, 0:N], t2p[:, N:2 * N])
                        nc.scalar.activation(lnt[:], t2[:], AF.Ln)
                        if sx == 0:
                            # dummy transpose keeps PE inside the ~3.4us HAM
                            # window through the pointwise tail (the real
                            # transpose overwrites the same range later)
                            nc.tensor.transpose(dr_ps[:, 0:P],
                                                t2p[:, 0:P], ident[:])
                        nc.scalar.activation(tm[:], lnt[:], AF.Exp, scale=0.5)
                        nc.vector.tensor_add(a12[:, 0:N], sps_s, tm[:])
                        nc.vector.tensor_sub(a12[:, N:2 * N], sps_s, tm[:])
                        nc.vector.tensor_mul(sq12[:], a12[:], a12[:])
                        nc.scalar.activation(c12[:], sq12[:], AF.Exp,
                                             scale=exp_scale)
                        nc.vector.tensor_mul(f12[:], c12[:], a12[:])
                        nc.gpsimd.tensor_add(delta[:], f12[:, 0:N],
                                             f12[:, N:2 * N])
                        # transpose delta col-half sx back to R layout
                        for b in range(NB):
                            nc.tensor.transpose(
                                dr_ps[:, N * b + P * sx:N * b + P * sx + P],
                                delta[:, P * b:P * b + P],
                                ident[:])
                        # u_{n+1} col-half sx = u_n + s*delta (from PSUM)
                        nc.vector.scalar_tensor_tensor(
                            halves(u_next[:])[sx], halves(dr_ps[:])[sx],
                            upd_scale, halves(u_t[:])[sx],
                            op0=OP.mult, op1=OP.add)
                    u_t = u_next

            nc.sync.dma_start(out_d, u_t[:])

    try:
        nc.compile()
    finally:
        bacc_mod.get_activation_tables = _orig_tables
    return nc


# ---------------------------------------------------------------- entry point
def _input_maps(u0, dt):
    wtt = _wt_tile(_T)
    wtd = _wt_tile(_D)
    wtg = _wt_tile(_G)
    ident = np.eye(P, dtype=np.float32)
    return [
        {"u0": _pack_img(u0[c]),
         "u0c2": _pack_img(np.ascontiguousarray(2.0 * u0[c].T)),
         "wtt": wtt, "wtd": wtd, "wtg": wtg, "ident": ident}
        for c in range(u0.shape[0])
    ]


def kernel(x, dt, k):
    from concourse.bass_utils import run_bass_kernel_spmd

    x = np.asarray(x, dtype=np.float32)
    dt_f = float(np.asarray(dt))
    k_f = float(np.asarray(k))
    B = x.shape[0]
    assert x.shape == (N_CORES, 3, N, N)

    u0 = (np.float32(_LUMA[0]) * x[:, 0] + np.float32(_LUMA[1]) * x[:, 1]
          + np.float32(_LUMA[2]) * x[:, 2]).astype(np.float32)

    nc = _build(dt_f, k_f, ITERS)

    in_maps = _input_maps(u0, dt_f)
    res = run_bass_kernel_spmd(nc, in_maps, core_ids=list(range(N_CORES)))
    u_fin = np.stack([_unpack_img(res.results[c]["uout"]) for c in range(B)])
    return np.repeat(u_fin[:, None, :, :], 3, axis=1).astype(np.float32)


# revision 16
# speedup vs baseline: 1.2204x; 1.2204x over previous
"""Coherence-enhancing diffusion layer on 8 TRN2 NeuronCores.

Data-parallel: one 256x256 image per core, 100 diffusion iterations fully
on-chip.  All-fp32 (the 100-iteration nonlinear diffusion chaotically
amplifies any lower-precision rounding; fp32r matmuls measure ~1.5e-4/pass
which amplifies to ~6e-2 final error -- over the gate).

Structure per iteration (u stored in "R" layout [128, 512]:
tile[p, 256*b + c] = u[128*b + p, c]):
  - Sobel column convs ([-1,0,1] / [1,2,1]) as DVE/Pool free-axis shifts.
  - Sobel row convs as banded 256x256 matmuls on PE -> gx^T, gy^T ("C").
  - Structure tensor products; rho never materializes (blur linearity:
    blur2(blur1(q) - blur1(c*u0)) = blur(q) - c*rho).
  - 3 separable 9-tap Gaussian blurs as banded matmul pass pairs; the
    B (=2*S12) and D (=S11-S22) pass-2 outputs are interleaved per
    column-half into shared PSUM banks [2B | D] so the tail squares both
    with ONE 512-wide ACT op; 4*B^2+D^2 = t^2 follows as one add.
  - Pointwise tail per column-half: a1/a2 written into adjacent halves of
    one [128,512] tile so square/exp/mul run as single 512-wide ops;
    sqrt via exp(0.5*ln(x)) keeps ACT on one LUT set (a real Sqrt forces
    a ~2.7us table swap twice/iter).
  - Engine balance: ACT ~6.8us, DVE ~8.4us, Pool ~7us, PE ~7.5us per
    iteration; scheduling overlap across the two column-half chains and
    across iterations (u updates gate per column-half).
  - delta transposed back to R via 4 PE transpose matmuls; the u update
    reads transposed delta straight from PSUM (also keeps PE warm through
    the pointwise tail).
"""
import numpy as np

N = 256
P = 128
NB = 2
ITERS = 100
RHO_INT = 4
KSIZE = 9
N_CORES = 8

_LUMA = (0.299, 0.587, 0.114)


# ---------------------------------------------------------------- host math
def _gauss1d_f32():
    half = (KSIZE - 1) / 2.0
    xs = np.linspace(-half, half, KSIZE).astype(np.float32)
    t = xs / np.float32(RHO_INT)
    pdf = np.exp(np.float32(-0.5) * t * t).astype(np.float32)
    return (pdf / pdf.sum()).astype(np.float32)


def _band_matrix(taps, pad):
    r = len(taps) // 2
    M = np.zeros((N, N), np.float64)
    for i in range(N):
        for d in range(-r, r + 1):
            j = i + d
            w = taps[d + r]
            if pad == "zero":
                if 0 <= j < N:
                    M[i, j] += w
            else:  # reflect (jnp.pad mode='reflect')
                if j < 0:
                    j = -j
                elif j >= N:
                    j = 2 * (N - 1) - j
                M[i, j] += w
    return M.astype(np.float32)


def _wt_tile(W):
    """Pack W^T into a [128, 512] block-major tile: out[p, 256k+n] = W[n, 128k+p]."""
    out = np.empty((P, NB * N), np.float32)
    for k in range(NB):
        out[:, N * k:N * (k + 1)] = W.T[P * k:P * (k + 1), :]
    return np.ascontiguousarray(out)


def _nz_ranges(W):
    """Per contraction block k: [lo, hi) range of output rows r' with any
    nonzero weight W[r', r], r in block k."""
    rngs = []
    for k in range(NB):
        nz = np.nonzero(np.any(W[:, P * k:P * (k + 1)] != 0, axis=1))[0]
        rngs.append((int(nz.min()), int(nz.max()) + 1))
    return rngs


def _pack_img(img):
    """[256,256] -> [128,512] R-layout."""
    return np.ascontiguousarray(
        img.reshape(NB, P, N).transpose(1, 0, 2).reshape(P, NB * N))


def _unpack_img(tile_arr):
    return tile_arr.reshape(P, NB, N).transpose(1, 0, 2).reshape(N, N)


_G = _band_matrix(_gauss1d_f32().astype(np.float64), "reflect")
_T = _band_matrix([1.0, 2.0, 1.0], "zero")
_D = _band_matrix([-1.0, 0.0, 1.0], "zero")


# ---------------------------------------------------------------- bass build
def _build(dt, k, iters, loop_m=1):
    import concourse.bass as bass  # noqa: F401
    import concourse.tile as tile
    from concourse import bacc, mybir

    F32 = mybir.dt.float32
    AF = mybir.ActivationFunctionType
    OP = mybir.AluOpType

    k2 = np.float32(k) * np.float32(k)
    exp_scale = float(-0.25 / k2)
    upd_scale = float(np.float32(0.5) * np.float32(dt))

    nc = bacc.Bacc("TRN2", target_bir_lowering=False, debug=False)

    # Pin all activations to the one LUT set that holds ln+exp+square+copy
    # ("natural_log_exp_and_others") so the act-table pass hoists a single
    # table load to kernel entry instead of ping-ponging sets (~2.7us each).
    import concourse.bacc as bacc_mod
    _orig_tables = bacc_mod.get_activation_tables

    def _pinned_tables(arch):
        tabs = _orig_tables(arch)
        return {
            name: (funcs if "natural_log_exp" in name else set())
            for name, funcs in tabs.items()
        }

    bacc_mod.get_activation_tables = _pinned_tables

    u0_d = nc.dram_tensor("u0", [P, NB * N], F32, kind="ExternalInput").ap()
    u0c2_d = nc.dram_tensor("u0c2", [P, NB * N], F32, kind="ExternalInput").ap()
    wtt_d = nc.dram_tensor("wtt", [P, NB * N], F32, kind="ExternalInput").ap()
    wtd_d = nc.dram_tensor("wtd", [P, NB * N], F32, kind="ExternalInput").ap()
    wtg_d = nc.dram_tensor("wtg", [P, NB * N], F32, kind="ExternalInput").ap()
    id_d = nc.dram_tensor("ident", [P, P], F32, kind="ExternalInput").ap()
    out_d = nc.dram_tensor("uout", [P, NB * N], F32, kind="ExternalOutput").ap()

    rng_t = _nz_ranges(_T)   # == ranges for D (same band)
    rng_g = _nz_ranges(_G)

    with tile.TileContext(nc) as tc:
        with (
            tc.tile_pool(name="consts", bufs=1) as consts,
            tc.tile_pool(name="upool", bufs=(6 if loop_m > 1 else 3)) as upool,
            tc.tile_pool(name="work", bufs=3) as work,
            tc.tile_pool(name="ps", bufs=8, space="PSUM") as psp,
        ):
            wtt = consts.tile([P, NB * N], F32)
            wtd = consts.tile([P, NB * N], F32)
            wtg = consts.tile([P, NB * N], F32)
            u0c2 = consts.tile([P, NB * N], F32)
            ident = consts.tile([P, P], F32)
            nc.sync.dma_start(wtt[:], wtt_d)
            nc.sync.dma_start(wtd[:], wtd_d)
            nc.sync.dma_start(wtg[:], wtg_d)
            nc.sync.dma_start(u0c2[:], u0c2_d)
            nc.sync.dma_start(ident[:], id_d)

            u_t = upool.tile([P, NB * N], F32, tag="u")
            nc.sync.dma_start(u_t[:], u0_d)
            m1u0d = consts.tile([P, NB * N], F32)

            def emit_band(x_tile, w_tile, rngs, out256, j):
                """(W @ X)^T section j -> out256 ([128,256] PSUM window):
                one 2-matmul accumulation group."""
                (lo0, hi0), (lo1, hi1) = rngs
                lhs0 = x_tile[:, N * 0 + P * j:N * 0 + P * j + P]
                lhs1 = x_tile[:, N * 1 + P * j:N * 1 + P * j + P]
                nc.tensor.matmul(out256[:, lo0:hi0], lhs0,
                                 w_tile[:, 0 + lo0:0 + hi0],
                                 start=True, stop=False)
                nc.tensor.matmul(out256[:, lo1:N], lhs1,
                                 w_tile[:, N + lo1:N + N],
                                 start=False, stop=True)

            def conv_pass(x_tile, w_tile, rngs, psum_tile):
                """psum = (W @ X)^T, both sections; one group per section."""
                for j in range(NB):
                    emit_band(x_tile, w_tile, rngs,
                              psum_tile[:, N * j:N * j + N], j)

            def seg(ap, a, b):
                """[128,512] tile -> AP [128, 2, b-a]: free cols a..b of both
                sections."""
                return ap.rearrange("p (s c) -> p s c", s=NB)[:, :, a:b]

            def halves(ap):
                return [seg(ap, P * h, P * h + P) for h in range(NB)]

            # one-time: m1u0d = blur1(2*u0c); rho subtractions ride on the
            # pass-1 PSUM->SBUF moves via blur linearity.
            m1u_ps = psp.tile([P, NB * N], F32, tag="ps")
            conv_pass(u0c2[:], wtg[:], rng_g, m1u_ps)
            nc.scalar.copy(m1u0d[:], m1u_ps[:])

            # ---- diffusion iterations ----
            import contextlib
            outer = tc.For_i(0, loop_m, 1) if loop_m > 1 else contextlib.nullcontext()
            with outer:
                for it in range(iters):
                    u = u_t[:]
                    # Sobel column convs (free-axis shifts, zero-pad):
                    # ud[c] = u[c+1] - u[c-1];  ut[c] = u[c-1] + 2u[c] + u[c+1]
                    # (full-width ops: by the time u's second half lands the
                    # engines are serialized on the u update anyway, and
                    # fewer instructions beat per-half gating here)
                    ud = work.tile([P, NB * N], F32)
                    s1 = work.tile([P, NB * N], F32)
                    ut = work.tile([P, NB * N], F32)
                    nc.vector.tensor_add(seg(s1[:], 0, N - 1),
                                         seg(u, 0, N - 1), seg(u, 1, N))
                    nc.gpsimd.tensor_sub(seg(ud[:], 1, N - 1),
                                         seg(u, 2, N), seg(u, 0, N - 2))
                    nc.vector.tensor_add(seg(ut[:], 1, N - 1),
                                         seg(s1[:], 0, N - 2),
                                         seg(s1[:], 1, N - 1))
                    # edges (zero pad): ud[0]=u[1]; ud[255]=-u[254];
                    # ut[0]=s1[0]+u[0]; ut[255]=s1[254]+u[255]
                    nc.gpsimd.tensor_copy(seg(ud[:], 0, 1), seg(u, 1, 2))
                    nc.gpsimd.tensor_scalar_mul(seg(ud[:], N - 1, N),
                                                seg(u, N - 2, N - 1), -1.0)
                    nc.vector.tensor_add(seg(ut[:], 0, 1), seg(s1[:], 0, 1),
                                         seg(u, 0, 1))
                    nc.vector.tensor_add(seg(ut[:], N - 1, N),
                                         seg(s1[:], N - 2, N - 1),
                                         seg(u, N - 1, N))

                    # Sobel row convs on PE -> gx^T, gy^T (C layout).
                    # PSUM dependencies are bank-granular, so each section
                    # gets its own bank: section-0 consumers start without
                    # waiting for section-1 matmuls.  gy first: q12 (the
                    # lead blur's input) waits on the gy_c copy.
                    gy_b = [psp.tile([P, N], F32, name=f"gyb{j}", tag="ps")
                            for j in range(NB)]
                    gx_b = [psp.tile([P, N], F32, name=f"gxb{j}", tag="ps")
                            for j in range(NB)]
                    for j in range(NB):
                        emit_band(ut[:], wtd[:], rng_t, gy_b[j][:], j)
                        emit_band(ud[:], wtt[:], rng_t, gx_b[j][:], j)

                    # structure tensor entries (C layout), per section
                    gy_c = work.tile([P, NB * N], F32)
                    q12 = work.tile([P, NB * N], F32)
                    q11 = work.tile([P, NB * N], F32)
                    q22 = work.tile([P, NB * N], F32)
                    mq = work.tile([P, NB * N], F32)
                    pq = work.tile([P, NB * N], F32)
                    for sx in range(NB):
                        ss = lambda t_: t_[:, N * sx:N * sx + N]
                        nc.scalar.copy(ss(gy_c), gy_b[sx][:])
                        nc.scalar.activation(ss(q11), gx_b[sx][:], AF.Square)
                        nc.vector.tensor_mul(ss(q12), gx_b[sx][:], ss(gy_c))
                        nc.scalar.activation(ss(q22), gy_b[sx][:], AF.Square)
                        nc.gpsimd.tensor_sub(ss(mq), ss(q11), ss(q22))
                        nc.gpsimd.tensor_add(ss(pq), ss(q11), ss(q22))

                    # blur pass 1 (B quantity first so its chain leads)
                    m1c_ps = psp.tile([P, NB * N], F32, tag="ps")
                    conv_pass(q12[:], wtg[:], rng_g, m1c_ps)
                    m1b_ps = psp.tile([P, NB * N], F32, tag="ps")
                    conv_pass(mq[:], wtg[:], rng_g, m1b_ps)
                    m1a_ps = psp.tile([P, NB * N], F32, tag="ps")
                    conv_pass(pq[:], wtg[:], rng_g, m1a_ps)
                    # full-width PSUM->SBUF moves with the rho folds:
                    # B path doubled (2*blur1(q12) - blur1(2u0c)) so the tail
                    # squares [2B | D] with a single op per section.
                    m1c_c = work.tile([P, NB * N], F32)
                    m1b_c = work.tile([P, NB * N], F32)
                    m1a_c = work.tile([P, NB * N], F32)
                    nc.vector.scalar_tensor_tensor(
                        m1c_c[:], m1c_ps[:], 2.0, m1u0d[:],
                        op0=OP.mult, op1=OP.subtract)
                    nc.scalar.copy(m1b_c[:], m1b_ps[:])
                    nc.vector.tensor_sub(m1a_c[:], m1a_ps[:], m1u0d[:])

                    # blur pass 2: section-major into shared [2B | D] banks
                    # (bd0 first so the section-0 tail chain starts earliest),
                    # then S into its own bank.
                    bd = [psp.tile([P, NB * N], F32, name=f"bd{j}", tag="ps")
                          for j in range(NB)]
                    s_b = [psp.tile([P, N], F32, name=f"sb{j}", tag="ps")
                           for j in range(NB)]
                    for j in range(NB):
                        emit_band(m1c_c[:], wtg[:], rng_g, bd[j][:, 0:N], j)
                        emit_band(m1b_c[:], wtg[:], rng_g, bd[j][:, N:2 * N], j)
                    for j in range(NB):
                        emit_band(m1a_c[:], wtg[:], rng_g, s_b[j][:], j)

                    # pointwise tail per column-half sx:
                    #   t^2 = (2B)^2 + D^2; t = exp(.5 ln t^2);
                    #   a12 = [S+t | S-t] (= 2*lam); c12 = exp(-a12^2/(4k^2));
                    #   delta = sum of c12*a12 halves; u += dt/2 * delta^T
                    dr = [psp.tile([P, N], F32, name=f"dr{j}", tag="ps")
                          for j in range(NB)]
                    u_next = upool.tile([P, NB * N], F32, tag="u")
                    for sx in range(NB):
                        t2p = work.tile([P, NB * N], F32, tag=f"t2p{sx}")
                        t2 = work.tile([P, N], F32, tag=f"t2{sx}")
                        lnt = work.tile([P, N], F32, tag=f"lnt{sx}")
                        tm = work.tile([P, N], F32, tag=f"tm{sx}")
                        a12 = work.tile([P, NB * N], F32, tag=f"a12{sx}")
                        sq12 = work.tile([P, NB * N], F32, tag=f"sq12{sx}")
                        c12 = work.tile([P, NB * N], F32, tag=f"c12{sx}")
                        f12 = work.tile([P, NB * N], F32, tag=f"f12{sx}")

                        nc.scalar.activation(t2p[:], bd[sx][:], AF.Square)
                        nc.gpsimd.tensor_add(t2[:], t2p[:, 0:N], t2p[:, N:2 * N])
                        nc.scalar.activation(lnt[:], t2[:], AF.Ln)
                        nc.scalar.activation(tm[:], lnt[:], AF.Exp, scale=0.5)
                        nc.vector.tensor_add(a12[:, 0:N], s_b[sx][:], tm[:])
                        nc.vector.tensor_sub(a12[:, N:2 * N], s_b[sx][:], tm[:])
                        nc.scalar.activation(sq12[:], a12[:], AF.Square)
                        if sx == 0:
                            # dummy transpose keeps PE inside the ~3.4us HAM
                            # window through the pointwise tail (the real
                            # transpose overwrites the same range later)
                            nc.tensor.transpose(dr[0][:, 0:P],
                                                a12[:, 0:P], ident[:])
                        nc.scalar.activation(c12[:], sq12[:], AF.Exp,
                                             scale=exp_scale)
                        nc.vector.tensor_mul(f12[:], c12[:], a12[:])
                        # delta = f1 + f2 folded into the transposes: both
                        # halves transpose-accumulate into the same PSUM
                        # range (start only on the first)
                        for b in range(NB):
                            nc.tensor.matmul(
                                dr[sx][:, P * b:P * b + P],
                                f12[:, P * b:P * b + P], ident[:],
                                is_transpose=True, start=True, stop=False)
                            nc.tensor.matmul(
                                dr[sx][:, P * b:P * b + P],
                                f12[:, N + P * b:N + P * b + P], ident[:],
                                is_transpose=True, start=False, stop=True)
                        # u_{n+1} col-half sx = u_n + s*delta (from PSUM)
                        nc.vector.scalar_tensor_tensor(
                            halves(u_next[:])[sx],
                            dr[sx][:].rearrange("p (b c) -> p b c", b=NB),
                            upd_scale, halves(u_t[:])[sx],
                            op0=OP.mult, op1=OP.add)
                    u_t = u_next

            nc.sync.dma_start(out_d, u_t[:])

    try:
        nc.compile()
    finally:
        bacc_mod.get_activation_tables = _orig_tables
    return nc


# ---------------------------------------------------------------- entry point
def _input_maps(u0, dt):
    wtt = _wt_tile(_T)
    wtd = _wt_tile(_D)
    wtg = _wt_tile(_G)
    ident = np.eye(P, dtype=np.float32)
    return [
        {"u0": _pack_img(u0[c]),
         "u0c2": _pack_img(np.ascontiguousarray(2.0 * u0[c].T)),
         "wtt": wtt, "wtd": wtd, "wtg": wtg, "ident": ident}
        for c in range(u0.shape[0])
    ]


def kernel(x, dt, k):
    from concourse.bass_utils import run_bass_kernel_spmd

    x = np.asarray(x, dtype=np.float32)
    dt_f = float(np.asarray(dt))
    k_f = float(np.asarray(k))
    B = x.shape[0]
    assert x.shape == (N_CORES, 3, N, N)

    u0 = (np.float32(_LUMA[0]) * x[:, 0] + np.float32(_LUMA[1]) * x[:, 1]
          + np.float32(_LUMA[2]) * x[:, 2]).astype(np.float32)

    nc = _build(dt_f, k_f, ITERS)

    in_maps = _input_maps(u0, dt_f)
    res = run_bass_kernel_spmd(nc, in_maps, core_ids=list(range(N_CORES)))
    u_fin = np.stack([_unpack_img(res.results[c]["uout"]) for c in range(B)])
    return np.repeat(u_fin[:, None, :, :], 3, axis=1).astype(np.float32)
